# revision 3
# baseline (speedup 1.0000x reference)
"""Non-local block (embedded-dot-product, softmax-free) Trainium2 kernel.

Reference computation:
    theta/phi/g = 1x1 conv projections of x [B,C,H,W] -> [B,Ci,N]
    f = (theta^T phi)/N  [B,N,N];  y = f @ g^T  [B,N,Ci]
    out = BN(W(y)) + x

Key algebraic transform: no softmax => (theta@phi)@g == theta@(phi@g).
S = phi_x @ g_x^T is only [Ci,Ci]; the N x N affinity is never formed.
FLOPs drop ~32x and the kernel becomes memory-bound (reads x once,
writes out once).

Sharding: data-parallel over batch, 2 samples per core on 8 cores.
Weight-side constants (BN fold, bias fold, 1/N scale) are precomputed on
the host. The projections that consume x run in fp32r (full-rate on the
PE, ~tf32 accuracy, no cast pass needed); the small S/y/W matmuls run in
bf16 produced for free by the PSUM->SBUF copies. The residual add reads
the original fp32 bits of x.
"""

import numpy as np
import ml_dtypes

import concourse.bass as bass
import concourse.mybir as mybir
import concourse.tile as tile
from concourse.bass_utils import run_bass_kernel_spmd

F32 = mybir.dt.float32
F32R = mybir.dt.float32r
BF16 = mybir.dt.bfloat16
NPBF16 = ml_dtypes.bfloat16
ADD = mybir.AluOpType.add
IDENT = mybir.ActivationFunctionType.Identity

B, C, N, CI = 16, 256, 4096, 128
NCORES = 8
BL = B // NCORES  # samples per core
EPS = 1e-5

PIECE = 1024  # x streams in column pieces of this width
NP = N // PIECE  # 4 pieces per channel-chunk
NT = N // 128  # 32 spatial tiles (phi/g projection granularity)
NF = N // 512  # 8 spatial chunks (theta / y / W granularity)
TPP = PIECE // 128  # pg tiles per piece
FPP = PIECE // 512  # 512-chunks per piece


# This walrus build rejects any instruction encoding more than one sync-wait.
# Tile freely emits multi-wait instructions, so post-process the finished
# module: excess waits move onto same-engine NOPs inserted just before the
# instruction (the engine blocks on each in turn — semantically identical).
def _split_multiwait(nc):
    n_split = 0
    for fn in nc.m.functions:
        for bb in fn.blocks:
            out = []
            for inst in bb.instructions:
                si = getattr(inst, "sync_info", None)
                if si is not None and si.on_wait and len(si.on_wait) > 1:
                    waits = list(si.on_wait)
                    si.on_wait = [waits[-1]]
                    for i, w in enumerate(waits[:-1]):
                        out.append(
                            mybir.InstNoOp(
                                name=f"{inst.name}-sw{i}",
                                engine=inst.engine,
                                sync_info=mybir.SyncInfo(on_wait=[w], on_update=[]),
                                bass_nofuse=True,
                            )
                        )
                    n_split += 1
                out.append(inst)
            bb.instructions[:] = out
    return n_split


_NC = {}


def build_nc(repeat=1, **opts):
    """Build the per-core Bass module. repeat>1 wraps the body in a device-side
    For_i loop (same data recomputed; used only for wall-clock slope timing).
    opts: experiment knobs (no_in, no_out, piece, stt_split)."""
    key = (repeat, tuple(sorted((k, tuple(v) if isinstance(v, list) else v)
                                for k, v in opts.items())))
    if key in _NC:
        return _NC[key]
    no_in = opts.get("no_in", False)
    no_out = opts.get("no_out", False)
    piece = opts.get("piece", 2048)
    stt_split = opts.get("stt_split", True)
    yt_dve = opts.get("yt_dve", False)
    no_tail = opts.get("no_tail", False)
    s_lag = opts.get("s_lag", 1)
    th_split = opts.get("th_split", False)
    in_eng = opts.get("in_eng", "scalar")
    out_eng = opts.get("out_eng", "sync")
    in_interleave = opts.get("in_interleave", True)
    no_pg = opts.get("no_pg", False)
    out_batch = opts.get("out_batch", 4)  # stt 512-chunks per output DMA
    psum_cfg = tuple(opts.get("psum_cfg", (3, 4, 1)))
    np_pieces = N // piece
    nc = bass.Bass()

    x_d = nc.declare_dram_parameter("x", [BL, C, N], F32R, isOutput=False)
    tw_d = nc.declare_dram_parameter("tw", [C, CI], F32R, isOutput=False)
    tb_d = nc.declare_dram_parameter("tb", [CI, 1], F32, isOutput=False)
    pgw_d = nc.declare_dram_parameter("pgw", [C, 2 * CI], F32R, isOutput=False)
    pgb_d = nc.declare_dram_parameter("pgb", [128, 2 * CI], F32, isOutput=False)
    ww_d = nc.declare_dram_parameter("ww", [CI, C], BF16, isOutput=False)
    wd_d = nc.declare_dram_parameter("wd", [128, 2], F32, isOutput=False)
    out_d = nc.declare_dram_parameter("out", [BL, C, N], F32, isOutput=True)

    with tile.TileContext(nc) as tc:
        with (
            tc.tile_pool(name="consts", bufs=1) as cpool,
            tc.tile_pool(name="xf", bufs=2 * BL) as xfp,
            tc.tile_pool(name="th", bufs=2) as thp,
            tc.tile_pool(name="pg", bufs=NT + 4) as pgp,
            tc.tile_pool(name="ssb", bufs=2) as ssbp,
            tc.tile_pool(name="yt", bufs=4) as ytp,
            tc.tile_pool(name="ob", bufs=6) as obp,
            tc.tile_pool(name="ps512", bufs=psum_cfg[0], space="PSUM") as ps512,
            tc.tile_pool(name="pgps", bufs=psum_cfg[1], space="PSUM") as pgps,
            tc.tile_pool(name="sps", bufs=psum_cfg[2], space="PSUM") as sps,
        ):
            # ---- constants into SBUF ----
            tw_sb = cpool.tile([128, 2, CI], F32R)
            for k in range(2):
                nc.sync.dma_start(tw_sb[:, k, :], tw_d[k * 128 : (k + 1) * 128, :])
            pgw_sb = cpool.tile([128, 2, 2 * CI], F32R)
            for k in range(2):
                nc.sync.dma_start(pgw_sb[:, k, :], pgw_d[k * 128 : (k + 1) * 128, :])
            pgb_sb = cpool.tile([128, 2 * CI], F32)
            nc.sync.dma_start(pgb_sb[:], pgb_d[:])
            tb_sb = cpool.tile([128, 1], F32)
            nc.sync.dma_start(tb_sb[:], tb_d[:])
            ww_sb = cpool.tile([128, C], BF16)
            nc.sync.dma_start(ww_sb[:], ww_d[:])
            wd_sb = cpool.tile([128, 2], F32)
            nc.sync.dma_start(wd_sb[:], wd_d[:])

            if no_in:
                xf_shared = [cpool.tile([128, N], F32R, name=f"xfc{c}") for c in range(2)]
                for t_ in xf_shared:
                    nc.vector.memset(t_[:].bitcast(F32), 0.5)
            if no_pg:
                s_shared = cpool.tile([128, CI], BF16, name="s_shared")
                nc.vector.memset(s_shared[:], 0.01)

            def _body():
                # all input DMAs issue first (SP never blocks input streaming
                # behind output-side waits); pieces release consumers early
                xfs = []
                for b in range(BL):
                    if no_in:
                        xfs.append(xf_shared)
                        continue
                    engs = {"sync": [nc.sync], "gp": [nc.gpsimd],
                            "mix": [nc.sync, nc.scalar],
                            "mix3": [nc.sync, nc.scalar, nc.gpsimd]}[in_eng]
                    di = 0
                    xf = [xfp.tile([128, N], F32R, name="xf_t", uniquify=True)
                          for _ in range(2)]
                    order = (
                        [(c, j) for j in range(np_pieces) for c in range(2)]
                        if in_interleave
                        else [(c, j) for c in range(2) for j in range(np_pieces)]
                    )
                    for c, j in order:
                        engs[di % len(engs)].dma_start(
                            xf[c][:, j * piece : (j + 1) * piece],
                            x_d[b, c * 128 : (c + 1) * 128,
                                j * piece : (j + 1) * piece],
                        )
                        di += 1
                    xfs.append(xf)

                for b in range(BL):
                    xf = xfs[b]

                    # phi/g projections + S accumulation + theta, streamed by
                    # x piece so compute starts as soon as data lands
                    pgt = []
                    th_sb = thp.tile([128, N], BF16, name="th_sb")
                    if not no_pg:
                        s_ps = sps.tile([128, CI], F32, name="s_ps")
                    tpp = piece // 128
                    fpp = piece // 512
                    for j in range(np_pieces):
                        for tj in ([] if no_pg else range(tpp)):
                            t = j * tpp + tj
                            pg_ps = pgps.tile([128, 2 * CI], F32, name="pg_ps")
                            for k in range(2):
                                nc.tensor.matmul(
                                    pg_ps[:],
                                    lhsT=xf[k][:, t * 128 : (t + 1) * 128],
                                    rhs=pgw_sb[:, k, :],
                                    start=(k == 0),
                                    stop=(k == 1),
                                )
                            pg_t = pgp.tile([128, 2 * CI], BF16, name="pg_t")
                            nc.vector.tensor_add(pg_t[:], pg_ps[:], pgb_sb[:])
                            pgt.append(pg_t)
                            # S^T matmul, s_lag tiles behind (DVE slack);
                            # lhsT=gT, rhs=phiT so psum = S^T = [c(g), j(phi)]
                            if t >= s_lag:
                                nc.tensor.matmul(
                                    s_ps[:],
                                    lhsT=pgt[t - s_lag][:, CI:],
                                    rhs=pgt[t - s_lag][:, :CI],
                                    start=(t - s_lag == 0),
                                    stop=False,
                                )
                        for fj in range(fpp):
                            f = j * fpp + fj
                            th_ps = ps512.tile([128, 512], F32, name="mm_ps")
                            for k in range(2):
                                nc.tensor.matmul(
                                    th_ps[:],
                                    lhsT=tw_sb[:, k, :],
                                    rhs=xf[k][:, f * 512 : (f + 1) * 512],
                                    start=(k == 0),
                                    stop=(k == 1),
                                )
                            if th_split and f % 2 == 1:
                                nc.vector.tensor_scalar_add(
                                    th_sb[:, f * 512 : (f + 1) * 512],
                                    th_ps[:],
                                    tb_sb[:],
                                )
                            else:
                                nc.scalar.activation(
                                    th_sb[:, f * 512 : (f + 1) * 512],
                                    th_ps[:],
                                    IDENT,
                                    bias=tb_sb[:],
                                )
                    if not no_pg:
                        for tt in range(NT - s_lag, NT):
                            nc.tensor.matmul(
                                s_ps[:],
                                lhsT=pgt[tt][:, CI:],
                                rhs=pgt[tt][:, :CI],
                                start=False,
                                stop=(tt == NT - 1),
                            )

                    if no_pg:
                        s_sb = s_shared
                    else:
                        s_sb = ssbp.tile([128, CI], BF16, name="s_sb")
                        nc.scalar.copy(s_sb[:], s_ps[:])

                    # ---- fold W into S: M^T[j, cout] = sum_c S^T[c,j] Weff^T[c,cout]
                    # (one [128,256] matmul), then w_y = M^T.T @ thetaT directly —
                    # the whole y intermediate never materializes
                    m_ps = ps512.tile([128, 512], F32, name="mm_ps")
                    nc.tensor.matmul(
                        m_ps[:, : 2 * CI],
                        lhsT=s_sb[:],
                        rhs=ww_sb[:],
                        start=True,
                        stop=True,
                    )
                    m_sb = ssbp.tile([128, 2 * CI], BF16, name="m_sb")
                    nc.vector.tensor_copy(m_sb[:], m_ps[:, : 2 * CI])

                    for f in ([] if no_tail else range(NF)):
                        for c in range(2):
                            w_ps = ps512.tile([128, 512], F32, name="mm_ps")
                            nc.tensor.matmul(
                                w_ps[:],
                                lhsT=m_sb[:, c * 128 : (c + 1) * 128],
                                rhs=th_sb[:, f * 512 : (f + 1) * 512],
                                start=True,
                                stop=True,
                            )
                            o_sb = obp.tile([128, 512], F32, name="o_sb")
                            if stt_split and c == 1:
                                # offload DVE: ACT adds D (psum->sbuf), GPSIMD
                                # adds the residual (sbuf-only)
                                wtmp = obp.tile([128, 512], F32, name="wtmp")
                                nc.scalar.activation(
                                    wtmp[:], w_ps[:], IDENT, bias=wd_sb[:, c : c + 1]
                                )
                                nc.gpsimd.tensor_add(
                                    o_sb[:], wtmp[:],
                                    xf[c][:, f * 512 : (f + 1) * 512].bitcast(F32),
                                )
                            else:
                                nc.vector.scalar_tensor_tensor(
                                    o_sb[:],
                                    in0=w_ps[:],
                                    scalar=wd_sb[:, c : c + 1],
                                    in1=xf[c][:, f * 512 : (f + 1) * 512].bitcast(F32),
                                    op0=ADD,
                                    op1=ADD,
                                )
                            if not no_out:
                                (nc.scalar if out_eng == "scalar" else nc.sync).dma_start(
                                    out_d[b, c * 128 : (c + 1) * 128,
                                          f * 512 : (f + 1) * 512],
                                    o_sb[:],
                                )

            if repeat == 1:
                _body()
            else:
                with tc.For_i(0, repeat, 1):
                    _body()

    _split_multiwait(nc)
    _NC[key] = nc
    return nc


def _host_consts(inputs):
    """Fold biases/BN on the host; returns per-core constant input arrays."""
    g_w = np.asarray(inputs["g_w"], np.float32)
    g_b = np.asarray(inputs["g_b"], np.float32)
    theta_w = np.asarray(inputs["theta_w"], np.float32)
    theta_b = np.asarray(inputs["theta_b"], np.float32)
    phi_w = np.asarray(inputs["phi_w"], np.float32)
    phi_b = np.asarray(inputs["phi_b"], np.float32)
    w_w = np.asarray(inputs["w_w"], np.float32)
    w_b = np.asarray(inputs["w_b"], np.float32)
    bn_gamma = np.asarray(inputs["bn_gamma"], np.float32)
    bn_beta = np.asarray(inputs["bn_beta"], np.float32)
    bn_mean = np.asarray(inputs["bn_mean"], np.float32)
    bn_var = np.asarray(inputs["bn_var"], np.float32)

    inv = bn_gamma / np.sqrt(bn_var + EPS)  # [C]
    tw = np.ascontiguousarray(theta_w.T).astype(np.float32)  # [C, CI]
    tb = theta_b.reshape(CI, 1).astype(np.float32)
    # fold 1/N into the g side
    gw_s = g_w / float(N)
    gb_s = g_b / float(N)
    pgw = np.ascontiguousarray(
        np.concatenate([phi_w.T, gw_s.T], axis=1)
    ).astype(np.float32)  # [C, 2Ci]
    pgb = np.tile(
        np.concatenate([phi_b, gb_s])[None, :], (128, 1)
    ).astype(np.float32)  # [128, 2Ci]
    ww = np.ascontiguousarray((w_w * inv[:, None]).T).astype(NPBF16)  # [CI, C]
    d = (w_b * inv + bn_beta - bn_mean * inv).astype(np.float32)  # [C]
    wd = np.ascontiguousarray(d.reshape(2, 128).T)  # [128, 2]
    return dict(tw=tw, tb=tb, pgw=pgw, pgb=pgb, ww=ww, wd=wd)


def device_inputs(inputs):
    """Full 8-core-stacked device input arrays, keyed by DRAM tensor name
    (axis 0 splits evenly across cores)."""
    x = np.ascontiguousarray(np.asarray(inputs["x"], np.float32)).reshape(B, C, N)
    consts = _host_consts(inputs)
    full = {"x": x}
    for k, v in consts.items():
        full[k] = np.concatenate([v] * NCORES, axis=0)
    return full


def kernel(**inputs):
    x = np.ascontiguousarray(np.asarray(inputs["x"], np.float32)).reshape(B, C, N)
    consts = _host_consts(inputs)
    nc = build_nc()
    in_maps = [
        {"x": np.ascontiguousarray(x[i * BL : (i + 1) * BL]), **consts}
        for i in range(NCORES)
    ]
    res = run_bass_kernel_spmd(nc, in_maps, core_ids=list(range(NCORES)))
    out = np.concatenate([r["out"] for r in res.results], axis=0)
    return out.reshape(B, C, 64, 64)



# revision 32
# speedup vs baseline: 19.8152x; 19.8152x over previous
"""Non-local block (embedded-dot-product, softmax-free) Trainium2 kernel.

Reference computation:
    theta/phi/g = 1x1 conv projections of x [B,C,H,W] -> [B,Ci,N]
    f = (theta^T phi)/N  [B,N,N];  y = f @ g^T  [B,N,Ci]
    out = BN(W(y)) + x

Key algebraic transform: no softmax => (theta@phi)@g == theta@(phi@g).
S = phi_x @ g_x^T is only [Ci,Ci]; the N x N affinity is never formed.
FLOPs drop ~32x and the kernel becomes memory-bound.

Sharding: data-parallel over batch, 2 samples per core on 8 cores.

Performance structure (measured on HW, see NTFF profiles):
- bf16 at the DRAM interface for x and out (io16): halves HBM traffic;
  total rel err ~3.5e-3 against the fp32 reference (gate is 2e-2).
- Few, large DMAs: each dma_start costs the issuing sequencer ~625ns and
  Tile's 8 DMAHW completion-sem lanes serialize on reuse. Inputs stream
  in graduated pieces [512, 1536, 2048] per chunk (small first piece
  beats the ~5us DMA-completion receipt latency to first compute);
  chunk0 rides the ACT HWDGE ring, chunk1 + outputs ride the SP ring.
- All weight-side constants (BN fold, bias fold, 1/N scale, host
  precomputed) arrive in ONE packed DMA, sliced/bitcast on device.
- phi/g projection pairs share a [128,512] PSUM tile and evict in one
  DVE op (PSUM evictions with per-column bias are DVE-only: GPSIMD
  can't read PSUM, ACT bias is per-partition).
- The two samples are software-pipelined: sample b+1's projection
  phase interleaves with sample b's W-tail so no engine queue
  head-of-line blocks (generator-based emission order).
"""

import numpy as np
import ml_dtypes

import concourse.bass as bass
import concourse.mybir as mybir
import concourse.tile as tile
from concourse.bass_utils import run_bass_kernel_spmd

F32 = mybir.dt.float32
F32R = mybir.dt.float32r
BF16 = mybir.dt.bfloat16
NPBF16 = ml_dtypes.bfloat16
ADD = mybir.AluOpType.add
IDENT = mybir.ActivationFunctionType.Identity

B, C, N, CI = 16, 256, 4096, 128
NCORES = 8
BL = B // NCORES  # samples per core
EPS = 1e-5
IO16 = True  # bf16 x/out at the DRAM interface (halves HBM traffic)

PIECE = 1024  # x streams in column pieces of this width
NP = N // PIECE  # 4 pieces per channel-chunk
NT = N // 128  # 32 spatial tiles (phi/g projection granularity)
NF = N // 512  # 8 spatial chunks (theta / y / W granularity)
TPP = PIECE // 128  # pg tiles per piece
FPP = PIECE // 512  # 512-chunks per piece


# This walrus build rejects any instruction encoding more than one sync-wait.
# Tile freely emits multi-wait instructions, so post-process the finished
# module: excess waits move onto same-engine NOPs inserted just before the
# instruction (the engine blocks on each in turn — semantically identical).
def _split_multiwait(nc):
    n_split = 0
    for fn in nc.m.functions:
        for bb in fn.blocks:
            out = []
            for inst in bb.instructions:
                si = getattr(inst, "sync_info", None)
                if si is not None and si.on_wait and len(si.on_wait) > 1:
                    waits = list(si.on_wait)
                    si.on_wait = [waits[-1]]
                    for i, w in enumerate(waits[:-1]):
                        out.append(
                            mybir.InstNoOp(
                                name=f"{inst.name}-sw{i}",
                                engine=inst.engine,
                                sync_info=mybir.SyncInfo(on_wait=[w], on_update=[]),
                                bass_nofuse=True,
                            )
                        )
                    n_split += 1
                out.append(inst)
            bb.instructions[:] = out
    return n_split


_NC = {}


def build_nc(repeat=1, **opts):
    """Build the per-core Bass module. repeat>1 wraps the body in a device-side
    For_i loop (same data recomputed; used only for wall-clock slope timing).
    opts: experiment knobs (no_in, no_out, piece, stt_split)."""
    key = (repeat, tuple(sorted((k, tuple(v) if isinstance(v, list) else v)
                                for k, v in opts.items())))
    if key in _NC:
        return _NC[key]
    no_in = opts.get("no_in", False)
    no_out = opts.get("no_out", False)
    piece = opts.get("piece", 2048)
    stt_split = opts.get("stt_split", True)
    yt_dve = opts.get("yt_dve", False)
    no_tail = opts.get("no_tail", False)
    s_lag = opts.get("s_lag", 1)
    th_split = opts.get("th_split", False)
    in_eng = opts.get("in_eng", "dual")
    out_eng = opts.get("out_eng", "sync")
    in_interleave = opts.get("in_interleave", True)
    no_pg = opts.get("no_pg", False)
    out_batch = opts.get("out_batch", 4)  # stt 512-chunks per output DMA
    io16 = opts.get("io16", IO16)
    ev_cycle = opts.get("ev_cycle", 3)  # unused (pg evicts are DVE-only)
    c1_add = opts.get("c1_add", "gp")  # tail c1 residual-add engine
    c0_act = opts.get("c0_act", 0)  # per sample: c0 chunks f>=NF-c0_act take
    # the ACT+add path instead of DVE-stt (DVE is the steady-phase pacer)
    psum_cfg = tuple(opts.get("psum_cfg", (3, 4, 1)))
    pieces = opts.get("pieces") or ([512, 1536, 2048] if piece == 2048 else [piece] * (N // piece))
    assert sum(pieces) == N and all(w % 256 == 0 for w in pieces)
    np_pieces = len(pieces)
    piece_offs = [sum(pieces[:i]) for i in range(np_pieces)]
    nc = bass.Bass()

    XDT = BF16 if io16 else F32R  # x/out DRAM + SBUF dtype
    ODT = BF16 if io16 else F32
    WDT = BF16 if io16 else F32R  # projection weights
    # all weight-side constants travel in ONE packed DMA (fewer DMAHW-lane
    # uses and less sequencer time); device slices views back out
    tw_w = CI if io16 else 2 * CI  # f32-words per partition
    pgw_w = 2 * CI if io16 else 4 * CI
    cst_w = tw_w + pgw_w + 4 * CI + 1 + C // 2 + 2  # pgb packed twice (pair evict)
    x_d = nc.declare_dram_parameter("x", [BL, C, N], XDT, isOutput=False)
    cst_d = nc.declare_dram_parameter("cst", [128, cst_w], F32, isOutput=False)
    out_d = nc.declare_dram_parameter("out", [BL, C, N], ODT, isOutput=True)

    with tile.TileContext(nc) as tc:
        with (
            tc.tile_pool(name="consts", bufs=1) as cpool,
            tc.tile_pool(name="xf", bufs=2 * BL) as xfp,
            tc.tile_pool(name="th", bufs=2) as thp,
            tc.tile_pool(name="pg", bufs=NT + 4) as pgp,
            tc.tile_pool(name="ssb", bufs=2) as ssbp,
            tc.tile_pool(name="yt", bufs=4) as ytp,
            tc.tile_pool(name="ob", bufs=6) as obp,
            tc.tile_pool(name="ps512", bufs=psum_cfg[0], space="PSUM") as ps512,
            tc.tile_pool(name="pgps", bufs=psum_cfg[1], space="PSUM") as pgps,
            tc.tile_pool(name="sps", bufs=psum_cfg[2], space="PSUM") as sps,
        ):
            # ---- constants into SBUF: one packed DMA, sliced views ----
            cst_sb = cpool.tile([128, cst_w], F32)
            nc.scalar.dma_start(cst_sb[:], cst_d[:])
            o = 0
            tw_v = cst_sb[:, o : o + tw_w].bitcast(WDT)  # [128, 2*CI]
            o += tw_w
            pgw_v = cst_sb[:, o : o + pgw_w].bitcast(WDT)  # [128, 2*2CI]
            o += pgw_w
            pgb2_sb = cst_sb[:, o : o + 4 * CI]  # [128, 2*2CI] f32 (pgb twice)
            pgb_sb = pgb2_sb[:, : 2 * CI]
            o += 4 * CI
            tb_sb = cst_sb[:, o : o + 1]  # [128, 1]
            o += 1
            ww_sb = cst_sb[:, o : o + C // 2].bitcast(BF16)  # [128, C]
            o += C // 2
            wd_sb = cst_sb[:, o : o + 2]  # [128, 2]

            if no_in:
                xf_shared = [cpool.tile([128, N], XDT, name=f"xfc{c}") for c in range(2)]
                for t_ in xf_shared:
                    nc.vector.memset(t_[:] if io16 else t_[:].bitcast(F32), 0.5)
            if no_pg:
                s_shared = cpool.tile([128, CI], BF16, name="s_shared")
                nc.vector.memset(s_shared[:], 0.01)

            def _body():
                # all input DMAs issue first (SP never blocks input streaming
                # behind output-side waits); pieces release consumers early
                xfs = []
                for b in range(BL):
                    if no_in:
                        xfs.append(xf_shared)
                        continue
                    engs = {"sync": [nc.sync], "gp": [nc.gpsimd],
                            "scalar": [nc.scalar], "dual": None,
                            "mix": [nc.sync, nc.scalar],
                            "mix3": [nc.sync, nc.scalar, nc.gpsimd]}[in_eng]
                    di = 0
                    xf = [xfp.tile([128, N], XDT, name="xf_t", uniquify=True)
                          for _ in range(2)]
                    order = (
                        [(c, j) for j in range(np_pieces) for c in range(2)]
                        if in_interleave
                        else [(c, j) for c in range(2) for j in range(np_pieces)]
                    )
                    for c, j in order:
                        # dual: chunk0 streams on the ACT ring, chunk1 on SP
                        # (both rings run concurrently; outputs queue on SP
                        # long after the inputs drain)
                        eng = ((nc.scalar if c == 0 else nc.sync)
                               if in_eng == "dual" else engs[di % len(engs)])
                        p0 = piece_offs[j]
                        eng.dma_start(
                            xf[c][:, p0 : p0 + pieces[j]],
                            x_d[b, c * 128 : (c + 1) * 128, p0 : p0 + pieces[j]],
                        )
                        di += 1
                    xfs.append(xf)

                def pg_phase(b, hook=None):
                    """Projections + S accumulation for sample b; returns the
                    state the tail needs. hook() is called at pair/chunk
                    boundaries to interleave the previous sample's tail."""
                    xf = xfs[b]
                    pgt = []
                    th_sb = thp.tile([128, N], BF16, name="th_sb")
                    if not no_pg:
                        s_ps = sps.tile([128, CI], F32, name="s_ps")
                    pg_pair_ps = None
                    next_f = 0
                    for j in range(np_pieces):
                        col_end = piece_offs[j] + pieces[j]
                        t0_, t1_ = piece_offs[j] // 128, col_end // 128
                        for t in ([] if no_pg else range(t0_, t1_)):
                            # two tiles share one [128, 512] PSUM pair and one
                            # wide eviction (halves eviction op count)
                            if t % 2 == 0:
                                pg_pair_ps = pgps.tile(
                                    [128, 4 * CI], F32, name="pg_ps"
                                )
                            pg_ps = pg_pair_ps[:, (t % 2) * 2 * CI :
                                               (t % 2 + 1) * 2 * CI]
                            for k in range(2):
                                nc.tensor.matmul(
                                    pg_ps,
                                    lhsT=xf[k][:, t * 128 : (t + 1) * 128],
                                    rhs=pgw_v[:, k * 2 * CI : (k + 1) * 2 * CI],
                                    start=(k == 0),
                                    stop=(k == 1),
                                )
                            if t % 2 == 1:
                                # PSUM-sourced: DVE only (GPSIMD can't read
                                # PSUM; ACT can't add the per-column bias)
                                pg_t2 = pgp.tile([128, 4 * CI], BF16, name="pg_t")
                                nc.vector.tensor_add(
                                    pg_t2[:], pg_pair_ps[:], pgb2_sb[:]
                                )
                                pgt.append(pg_t2[:, : 2 * CI])
                                pgt.append(pg_t2[:, 2 * CI :])
                            # S^T matmul, s_lag pairs behind (evict slack);
                            # lhsT=gT, rhs=phiT so psum = S^T = [c(g), j(phi)]
                            for ts in ([] if t % 2 == 0 else
                                       range(max(0, t - 1 - 2 * s_lag),
                                             max(0, t + 1 - 2 * s_lag))):
                                nc.tensor.matmul(
                                    s_ps[:],
                                    lhsT=pgt[ts][:, CI:],
                                    rhs=pgt[ts][:, :CI],
                                    start=(ts == 0),
                                    stop=False,
                                )
                            if t % 2 == 1 and hook:
                                hook()
                        while (next_f + 1) * 512 <= col_end:
                            f = next_f
                            next_f += 1
                            th_ps = ps512.tile([128, 512], F32, name="mm_ps")
                            for k in range(2):
                                nc.tensor.matmul(
                                    th_ps[:],
                                    lhsT=tw_v[:, k * CI : (k + 1) * CI],
                                    rhs=xf[k][:, f * 512 : (f + 1) * 512],
                                    start=(k == 0),
                                    stop=(k == 1),
                                )
                            if th_split and f % 2 == 1:
                                nc.vector.tensor_scalar_add(
                                    th_sb[:, f * 512 : (f + 1) * 512],
                                    th_ps[:],
                                    tb_sb[:],
                                )
                            else:
                                nc.scalar.activation(
                                    th_sb[:, f * 512 : (f + 1) * 512],
                                    th_ps[:],
                                    IDENT,
                                    bias=tb_sb[:],
                                )
                            if hook:
                                hook()
                    if not no_pg:
                        for tt in range(NT - 2 * s_lag, NT):
                            nc.tensor.matmul(
                                s_ps[:],
                                lhsT=pgt[tt][:, CI:],
                                rhs=pgt[tt][:, :CI],
                                start=(tt == 0),
                                stop=(tt == NT - 1),
                            )

                    if no_pg:
                        s_sb = s_shared
                    else:
                        s_sb = ssbp.tile([128, CI], BF16, name="s_sb")
                        nc.scalar.copy(s_sb[:], s_ps[:])

                    # ---- fold W into S: M^T[j, cout] = sum_c S^T[c,j] Weff^T[c,cout]
                    # (one [128,256] matmul), then w_y = M^T.T @ thetaT directly —
                    # the whole y intermediate never materializes
                    m_ps = ps512.tile([128, 512], F32, name="mm_ps")
                    nc.tensor.matmul(
                        m_ps[:, : 2 * CI],
                        lhsT=s_sb[:],
                        rhs=ww_sb[:],
                        start=True,
                        stop=True,
                    )
                    m_sb = ssbp.tile([128, 2 * CI], BF16, name="m_sb")
                    nc.vector.tensor_copy(m_sb[:], m_ps[:, : 2 * CI])
                    return dict(xf=xf, th_sb=th_sb, m_sb=m_sb)

                def tail_steps(b, st):
                    """Generator: W-matmul + bias/residual + output DMA for
                    sample b, one (f, c) chunk per yield."""
                    xf, th_sb, m_sb = st["xf"], st["th_sb"], st["m_sb"]
                    out_e = {"sync": nc.sync, "scalar": nc.scalar,
                             "gp": nc.gpsimd}[out_eng]
                    o_wide = [None, None]
                    for f in ([] if no_tail else range(NF)):
                        for c in range(2):
                            w_ps = ps512.tile([128, 512], F32, name="mm_ps")
                            nc.tensor.matmul(
                                w_ps[:],
                                lhsT=m_sb[:, c * 128 : (c + 1) * 128],
                                rhs=th_sb[:, f * 512 : (f + 1) * 512],
                                start=True,
                                stop=True,
                            )
                            if f % out_batch == 0:
                                o_wide[c] = obp.tile(
                                    [128, out_batch * 512], ODT,
                                    name=f"ow{c}", uniquify=True,
                                    tag="ow", bufs=4,
                                )
                            o_sb = o_wide[c][:, (f % out_batch) * 512 :
                                             (f % out_batch + 1) * 512]
                            if stt_split and (c == 1 or f >= NF - c0_act):
                                # offload DVE: ACT adds D (psum->sbuf), GPSIMD
                                # adds the residual (sbuf-only)
                                wtmp = obp.tile([128, 512], F32, name="wtmp")
                                nc.scalar.activation(
                                    wtmp[:], w_ps[:], IDENT, bias=wd_sb[:, c : c + 1]
                                )
                                add_e = {"gp": nc.gpsimd, "dve": nc.vector,
                                         "mix": (nc.gpsimd if f % 2 else nc.vector)
                                         }[c1_add]
                                add_e.tensor_add(
                                    o_sb, wtmp[:],
                                    xf[c][:, f * 512 : (f + 1) * 512]
                                    if io16 else
                                    xf[c][:, f * 512 : (f + 1) * 512].bitcast(F32),
                                )
                            else:
                                nc.vector.scalar_tensor_tensor(
                                    o_sb,
                                    in0=w_ps[:],
                                    scalar=wd_sb[:, c : c + 1],
                                    in1=xf[c][:, f * 512 : (f + 1) * 512]
                                    if io16 else
                                    xf[c][:, f * 512 : (f + 1) * 512].bitcast(F32),
                                    op0=ADD,
                                    op1=ADD,
                                )
                            if not no_out and f % out_batch == out_batch - 1:
                                g0 = (f - (out_batch - 1)) * 512
                                out_e.dma_start(
                                    out_d[b, c * 128 : (c + 1) * 128,
                                          g0 : (f + 1) * 512],
                                    o_wide[c][:],
                                )
                            yield

                # software pipeline: sample b+1's projection phase interleaves
                # with sample b's tail so no engine queue head-of-line blocks
                prev_tail = None
                states = []
                for b in range(BL):
                    def _hook():
                        if prev_tail is not None:
                            next(prev_tail, None)
                    states.append(pg_phase(b, hook=_hook if b > 0 else None))
                    if prev_tail is not None:
                        for _ in prev_tail:
                            pass
                    prev_tail = tail_steps(b, states[-1])
                for _ in prev_tail:
                    pass

            if repeat == 1:
                _body()
            else:
                with tc.For_i(0, repeat, 1):
                    _body()

    _split_multiwait(nc)
    _NC[key] = nc
    return nc


def _host_consts(inputs):
    """Fold biases/BN on the host; returns per-core constant input arrays."""
    g_w = np.asarray(inputs["g_w"], np.float32)
    g_b = np.asarray(inputs["g_b"], np.float32)
    theta_w = np.asarray(inputs["theta_w"], np.float32)
    theta_b = np.asarray(inputs["theta_b"], np.float32)
    phi_w = np.asarray(inputs["phi_w"], np.float32)
    phi_b = np.asarray(inputs["phi_b"], np.float32)
    w_w = np.asarray(inputs["w_w"], np.float32)
    w_b = np.asarray(inputs["w_b"], np.float32)
    bn_gamma = np.asarray(inputs["bn_gamma"], np.float32)
    bn_beta = np.asarray(inputs["bn_beta"], np.float32)
    bn_mean = np.asarray(inputs["bn_mean"], np.float32)
    bn_var = np.asarray(inputs["bn_var"], np.float32)

    inv = bn_gamma / np.sqrt(bn_var + EPS)  # [C]
    tw = np.ascontiguousarray(theta_w.T).astype(np.float32)  # [C, CI]
    tb = theta_b.reshape(CI, 1).astype(np.float32)
    # fold 1/N into the g side
    gw_s = g_w / float(N)
    gb_s = g_b / float(N)
    pgw = np.ascontiguousarray(
        np.concatenate([phi_w.T, gw_s.T], axis=1)
    ).astype(np.float32)  # [C, 2Ci]
    pgb = np.tile(
        np.concatenate([phi_b, gb_s])[None, :], (128, 1)
    ).astype(np.float32)  # [128, 2Ci]
    ww = np.ascontiguousarray((w_w * inv[:, None]).T).astype(NPBF16)  # [CI, C]
    d = (w_b * inv + bn_beta - bn_mean * inv).astype(np.float32)  # [C]
    wd = np.ascontiguousarray(d.reshape(2, 128).T)  # [128, 2]
    return dict(tw=tw, tb=tb, pgw=pgw, pgb=pgb, ww=ww, wd=wd)


def _pack_consts(consts, io16):
    """Pack all weight-side constants into one [128, words] f32 blob matching
    the device-side view layout in build_nc."""
    def as_bytes(a, np_dt):
        b = np.ascontiguousarray(a.astype(np_dt)).view(np.uint8).reshape(128, -1)
        pad = (-b.shape[1]) % 4
        if pad:
            b = np.concatenate([b, np.zeros((128, pad), np.uint8)], axis=1)
        return b

    wdt = NPBF16 if io16 else np.float32
    tw_p = consts["tw"].reshape(2, 128, CI).transpose(1, 0, 2).reshape(128, -1)
    pgw_p = consts["pgw"].reshape(2, 128, 2 * CI).transpose(1, 0, 2).reshape(128, -1)
    blob = np.concatenate(
        [
            as_bytes(tw_p, wdt),
            as_bytes(pgw_p, wdt),
            as_bytes(np.concatenate([consts["pgb"], consts["pgb"]], axis=1), np.float32),
            as_bytes(consts["tb"].reshape(128, 1), np.float32),
            as_bytes(consts["ww"], NPBF16),
            as_bytes(consts["wd"], np.float32),
        ],
        axis=1,
    )
    return np.ascontiguousarray(blob).view(np.float32)


def device_inputs(inputs, io16=None):
    """Full 8-core-stacked device input arrays, keyed by DRAM tensor name
    (axis 0 splits evenly across cores)."""
    io16 = IO16 if io16 is None else io16
    x = np.ascontiguousarray(np.asarray(inputs["x"], np.float32)).reshape(B, C, N)
    consts = _host_consts(inputs)
    if io16:
        x = x.astype(NPBF16)
    cst = _pack_consts(consts, io16)
    return {"x": x, "cst": np.concatenate([cst] * NCORES, axis=0)}


def percore_inputs(inputs, io16=None):
    full = device_inputs(inputs, io16)
    return [
        {
            k: np.ascontiguousarray(
                v[i * (v.shape[0] // NCORES) : (i + 1) * (v.shape[0] // NCORES)]
            )
            for k, v in full.items()
        }
        for i in range(NCORES)
    ]


def kernel(**inputs):
    nc = build_nc()
    in_maps = percore_inputs(inputs)
    res = run_bass_kernel_spmd(nc, in_maps, core_ids=list(range(NCORES)))
    out = np.concatenate([r["out"] for r in res.results], axis=0)
    return np.asarray(out, np.float32).reshape(B, C, 64, 64)

